# revision 9
# baseline (speedup 1.0000x reference)
import sys

sys.path.insert(0, "/opt/trn_rl_repo")
from contextlib import ExitStack

import numpy as np

from concourse import bacc, mybir
import concourse.bass as bass
import concourse.tile as tile
from concourse.bass_utils import run_bass_kernel_spmd

N = 100000
D = 128
NB = 782  # node blocks of 128
NPAD = NB * 128  # 100096
NCORES = 8
G = 8  # tiles batched per DVE op / gather ring group
EPS = 1e-8

_LAST_NC = None  # cached compiled program (reused by test harness for profiling)


def _build_metadata(edge_index: np.ndarray):
    """Per-core gather indices + local-target ids, block-sorted and padded.

    Returns (tb, per_core): tb[b] = tiles for target block b (same across
    cores); per_core dicts hold idx_oth [128, T] int32 and locf [128, T]
    float32 (local target id 0..127, -1 for padding).
    """
    src = edge_index[0].astype(np.int64)
    dst = edge_index[1].astype(np.int64)
    tgt = np.concatenate([src, dst])
    oth = np.concatenate([dst, src])
    blk = tgt >> 7

    order = np.argsort(blk, kind="stable")
    tgt_s, oth_s = tgt[order], oth[order]
    cnt_b = np.bincount(blk[order], minlength=NB)
    starts = np.zeros(NB + 1, np.int64)
    np.cumsum(cnt_b, out=starts[1:])

    # within each block, contribution i goes to core i % NCORES
    tb = np.ceil(np.ceil(cnt_b / NCORES) / 128).astype(np.int64)
    tb = np.maximum(tb, 1)
    T = int(tb.sum())
    tile_off = np.zeros(NB + 1, np.int64)
    np.cumsum(tb, out=tile_off[1:])

    per_core = []
    for c in range(NCORES):
        idx_oth = np.zeros((T, 128), np.int32)
        locf = np.full((T, 128), -1.0, np.float32)
        for b in range(NB):
            s, e = starts[b], starts[b + 1]
            sel = np.arange(s + c, e, NCORES)
            n = sel.size
            t0 = tile_off[b]
            rows = t0 + np.arange(n) // 128
            lanes = np.arange(n) % 128
            idx_oth[rows, lanes] = oth_s[sel]
            locf[rows, lanes] = (tgt_s[sel] & 127).astype(np.float32)
        per_core.append(
            {
                "idx_oth": np.ascontiguousarray(idx_oth.T),
                "locf": np.ascontiguousarray(locf.T),
            }
        )
    return tb, per_core


def _build_program(tb: np.ndarray):
    T = int(tb.sum())
    nc = bacc.Bacc("TRN2", target_bir_lowering=False, debug=False)
    feats = nc.dram_tensor("feats", [NPAD, D], mybir.dt.float32, kind="ExternalInput")
    idx_oth = nc.dram_tensor("idx_oth", [128, T], mybir.dt.int32, kind="ExternalInput")
    locf = nc.dram_tensor("locf", [128, T], mybir.dt.float32, kind="ExternalInput")
    iota = nc.dram_tensor("iota", [128, 1, 128], mybir.dt.float32, kind="ExternalInput")
    sums_out = nc.dram_tensor("sums_out", [128, NB], mybir.dt.float32, kind="ExternalOutput")

    # block id per tile, in order; first/last tile flags per block
    blk_of_tile = np.repeat(np.arange(NB), tb)
    first = np.zeros(T, bool)
    last = np.zeros(T, bool)
    off = np.zeros(NB + 1, np.int64)
    np.cumsum(tb, out=off[1:])
    first[off[:-1]] = True
    last[off[1:] - 1] = True

    with tile.TileContext(nc) as tc, ExitStack() as ctx:
        sb = ctx.enter_context(tc.tile_pool(name="sb", bufs=1))
        dram = ctx.enter_context(tc.tile_pool(name="dram", bufs=1, space="DRAM"))
        psum = ctx.enter_context(tc.tile_pool(name="psum", bufs=1, space="PSUM"))

        u_dram = dram.tile([NPAD, D], mybir.dt.float32)

        idx_oth_t = sb.tile([128, T], mybir.dt.int32)
        locf_t = sb.tile([128, T], mybir.dt.float32)
        iota_t = sb.tile([128, 1, 128], mybir.dt.float32)
        nc.sync.dma_start(idx_oth_t[:], idx_oth[:])
        nc.sync.dma_start(locf_t[:], locf[:])
        nc.sync.dma_start(iota_t[:], iota[:])

        # ---- phase 1: normalize features into u_dram ----
        NGRP = 2  # ring depth for norm chunks
        GN = 8  # blocks per norm chunk
        fring = [sb.tile([128, GN, D], mybir.dt.float32, name=f"fring{i}") for i in range(NGRP)]
        uring = [sb.tile([128, GN, D], mybir.dt.float32, name=f"uring{i}") for i in range(NGRP)]
        ssring = [sb.tile([128, GN], mybir.dt.float32, name=f"ssring{i}") for i in range(NGRP)]
        n_chunks = NB // GN
        rem = NB - n_chunks * GN
        for ci in range(n_chunks + (1 if rem else 0)):
            g0 = ci * GN
            gn = GN if ci < n_chunks else rem
            ft = fring[ci % NGRP]
            ut = uring[ci % NGRP]
            ss = ssring[ci % NGRP]
            for g in range(gn):
                r0 = (g0 + g) * 128
                nc.sync.dma_start(ft[:, g, :], feats[r0 : r0 + 128, :])
            nc.vector.tensor_tensor(
                out=ut[:, :gn, :], in0=ft[:, :gn, :], in1=ft[:, :gn, :], op=mybir.AluOpType.mult
            )
            nc.vector.tensor_reduce(
                out=ss[:, :gn], in_=ut[:, :gn, :], op=mybir.AluOpType.add,
                axis=mybir.AxisListType.X,
            )
            nc.scalar.sqrt(out=ss[:, :gn], in_=ss[:, :gn])
            nc.vector.tensor_scalar_max(ss[:, :gn], ss[:, :gn], EPS)
            nc.vector.reciprocal(out=ss[:, :gn], in_=ss[:, :gn])
            nc.vector.tensor_tensor(
                out=ut[:, :gn, :], in0=ft[:, :gn, :],
                in1=ss[:, :gn].unsqueeze(2).to_broadcast([128, gn, D]),
                op=mybir.AluOpType.mult,
            )
            for g in range(gn):
                r0 = (g0 + g) * 128
                nc.sync.dma_start(u_dram[r0 : r0 + 128, :], ut[:, g, :])

        # ---- phase 2: gather neighbors, one-hot matmul into per-block agg,
        #      then sums_b = rowwise_dot(agg_b, U_b) ----
        NPS = 6  # psum agg ring depth
        pagg = [psum.tile([128, D], mybir.dt.float32, name=f"pagg{i}") for i in range(NPS)]

        NRING = 3
        gO = [sb.tile([128, G, D], mybir.dt.float32, name=f"gO{i}") for i in range(NRING)]
        sel = [sb.tile([128, G, 128], mybir.dt.float32, name=f"sel{i}") for i in range(NRING)]

        NUB = 2
        GB = 8  # blocks per U_b load group
        ub = [sb.tile([128, GB, D], mybir.dt.float32, name=f"ub{i}") for i in range(NUB)]
        NPR = 2
        prod = [sb.tile([128, D], mybir.dt.float32, name=f"prod{i}") for i in range(NPR)]
        sums_sb = sb.tile([128, NB], mybir.dt.float32)

        def load_ub_group(bg):
            b0 = bg * GB
            bn = min(GB, NB - b0)
            t_ = ub[bg % NUB]
            for g in range(bn):
                r0 = (b0 + g) * 128
                nc.sync.dma_start(t_[:, g, :], u_dram[r0 : r0 + 128, :])

        load_ub_group(0)

        ngrp = (T + G - 1) // G
        for gi in range(ngrp):
            t0 = gi * G
            gn = min(G, T - t0)
            r = gi % NRING
            for g in range(gn):
                t = t0 + g
                nc.gpsimd.indirect_dma_start(
                    out=gO[r][:, g, :], out_offset=None, in_=u_dram[:],
                    in_offset=bass.IndirectOffsetOnAxis(ap=idx_oth_t[:, t : t + 1], axis=0),
                )
            nc.vector.tensor_tensor(
                out=sel[r][:, :gn, :],
                in0=locf_t[:, t0 : t0 + gn].unsqueeze(2).to_broadcast([128, gn, 128]),
                in1=iota_t[:].to_broadcast([128, gn, 128]),
                op=mybir.AluOpType.is_equal,
            )
            for g in range(gn):
                t = t0 + g
                b = int(blk_of_tile[t])
                nc.tensor.matmul(
                    out=pagg[b % NPS][:],
                    lhsT=sel[r][:, g, :],
                    rhs=gO[r][:, g, :],
                    start=bool(first[t]),
                    stop=bool(last[t]),
                )
                if last[t]:
                    # prefetch next U_b group when crossing a group boundary
                    if b % GB == GB - 1 and (b + 1) // GB < (NB + GB - 1) // GB:
                        load_ub_group((b + 1) // GB)
                    pr = prod[b % NPR]
                    nc.vector.tensor_tensor(
                        out=pr[:], in0=pagg[b % NPS][:], in1=ub[(b // GB) % NUB][:, b % GB, :],
                        op=mybir.AluOpType.mult,
                    )
                    nc.vector.tensor_reduce(
                        out=sums_sb[:, b : b + 1], in_=pr[:].unsqueeze(1),
                        op=mybir.AluOpType.add, axis=mybir.AxisListType.X,
                    )

        nc.sync.dma_start(sums_out[:], sums_sb[:])

    nc.compile()
    return nc


def kernel(node_features: np.ndarray, edge_index: np.ndarray) -> np.ndarray:
    feats = np.zeros((NPAD, D), np.float32)
    feats[:N] = np.asarray(node_features, np.float32)

    tb, per_core = _build_metadata(np.asarray(edge_index))
    nc = _build_program(tb)
    global _LAST_NC
    _LAST_NC = nc

    iota = np.broadcast_to(np.arange(128, dtype=np.float32)[None, None, :], (128, 1, 128))
    iota = np.ascontiguousarray(iota)
    in_maps = [
        {
            "feats": feats,
            "idx_oth": pc["idx_oth"],
            "locf": pc["locf"],
            "iota": iota,
        }
        for pc in per_core
    ]
    res = run_bass_kernel_spmd(nc, in_maps, core_ids=list(range(NCORES)), trace=False)

    sums = np.zeros((128, NB), np.float64)
    for out in res.results:
        sums += out["sums_out"].astype(np.float64)
    sums_nodes = sums.T.ravel()[:N]

    tgt = np.concatenate([edge_index[0], edge_index[1]]).astype(np.int64)
    deg = np.bincount(tgt, minlength=N).astype(np.float64)
    out = np.where(deg > 0, sums_nodes / np.maximum(deg, 1.0), 1.0)
    return out.astype(np.float32)


# revision 11
# speedup vs baseline: 1.4583x; 1.4583x over previous
import sys

sys.path.insert(0, "/opt/trn_rl_repo")
from contextlib import ExitStack

import numpy as np

from concourse import bacc, mybir
import concourse.bass as bass
import concourse.tile as tile
from concourse.bass_utils import run_bass_kernel_spmd

N = 100000
D = 128
NB = 782  # node blocks of 128
NPAD = NB * 128  # 100096
NCORES = 8
G = 8  # tiles per gather/compute group
GB = 8  # blocks per U_b load group
EPS = 1e-8

_LAST_NC = None  # cached compiled program (reused by test harness for profiling)


def _build_metadata(edge_index: np.ndarray):
    """Straddle-packed per-core gather metadata.

    Contributions (2 per edge) are sorted by target block and packed
    densely into 128-slot tiles that may straddle block boundaries.
    Per-block slot counts are padded to a multiple of NCORES so the
    layout (and thus the compiled program) is identical on every core.

    Returns (meta, per_core). meta: dict with T, segs [(tile, blk,
    first, last) sorted by tile], grp (per-G-tile-group segment ranges).
    per_core: idx_oth [128, T] int32, locf [128, S] float32 (-1 pad).
    """
    src = edge_index[0].astype(np.int64)
    dst = edge_index[1].astype(np.int64)
    tgt = np.concatenate([src, dst])
    oth = np.concatenate([dst, src])
    blk = tgt >> 7

    order = np.argsort(blk, kind="stable")
    tgt_s, oth_s = tgt[order], oth[order]
    loc_s = (tgt_s & 127).astype(np.float32)
    cnt_b = np.bincount(blk[order], minlength=NB)
    starts = np.zeros(NB + 1, np.int64)
    np.cumsum(cnt_b, out=starts[1:])

    q = -(-cnt_b // NCORES)  # per-core slots per block (ceil)
    Q = np.zeros(NB + 1, np.int64)
    np.cumsum(q, out=Q[1:])
    M = int(Q[-1])
    T = (M + 127) // 128
    MP = T * 128

    blk_slot = np.full(MP, -1, np.int64)
    for b in range(NB):
        blk_slot[Q[b] : Q[b + 1]] = b

    segs = []
    for b in range(NB):
        if q[b] == 0:
            continue
        t_lo = Q[b] // 128
        t_hi = (Q[b + 1] - 1) // 128
        for t in range(t_lo, t_hi + 1):
            segs.append((int(t), int(b), t == t_lo, t == t_hi))
    segs.sort()
    S = len(segs)

    ngrp = (T + G - 1) // G
    seg_tiles = np.array([s[0] for s in segs])
    grp = []
    for gi in range(ngrp):
        s0 = int(np.searchsorted(seg_tiles, gi * G, side="left"))
        s1 = int(np.searchsorted(seg_tiles, gi * G + G - 1, side="right"))
        grp.append((s0, s1))

    per_core = []
    for c in range(NCORES):
        idx_all = np.zeros(MP, np.int32)
        loc_all = np.full(MP, -1.0, np.float32)
        for b in range(NB):
            s, e = starts[b], starts[b + 1]
            sel = np.arange(s + c, e, NCORES)
            n = sel.size
            p0 = Q[b]
            idx_all[p0 : p0 + n] = oth_s[sel]
            loc_all[p0 : p0 + n] = loc_s[sel]
        locf_seg = np.full((S, 128), -1.0, np.float32)
        for s_i, (t, b, _, _) in enumerate(segs):
            sl = slice(t * 128, (t + 1) * 128)
            m = blk_slot[sl] == b
            locf_seg[s_i][m] = loc_all[sl][m]
        per_core.append(
            {
                "idx_oth": np.ascontiguousarray(idx_all.reshape(T, 128).T),
                "locf": np.ascontiguousarray(locf_seg.T),
            }
        )
    meta = {"T": T, "segs": segs, "grp": grp}
    return meta, per_core


def _build_program(meta):
    T, segs, grp = meta["T"], meta["segs"], meta["grp"]
    S = len(segs)
    msel = max((s1 - s0) for s0, s1 in grp)

    nc = bacc.Bacc("TRN2", target_bir_lowering=False, debug=False)
    feats = nc.dram_tensor("feats", [NPAD, D], mybir.dt.float32, kind="ExternalInput")
    idx_oth = nc.dram_tensor("idx_oth", [128, T], mybir.dt.int32, kind="ExternalInput")
    locf = nc.dram_tensor("locf", [128, S], mybir.dt.float32, kind="ExternalInput")
    iota = nc.dram_tensor("iota", [128, 1, 128], mybir.dt.float32, kind="ExternalInput")
    sums_out = nc.dram_tensor("sums_out", [128, NB], mybir.dt.float32, kind="ExternalOutput")

    with tile.TileContext(nc) as tc, ExitStack() as ctx:
        sb = ctx.enter_context(tc.tile_pool(name="sb", bufs=1))
        psum = ctx.enter_context(tc.tile_pool(name="psum", bufs=1, space="PSUM"))

        idx_oth_t = sb.tile([128, T], mybir.dt.int32)
        locf_t = sb.tile([128, S], mybir.dt.float32)
        iota_t = sb.tile([128, 1, 128], mybir.dt.float32)
        nc.sync.dma_start(idx_oth_t[:], idx_oth[:])
        nc.sync.dma_start(locf_t[:], locf[:])
        nc.sync.dma_start(iota_t[:], iota[:])

        NPS = 6
        pagg = [psum.tile([128, D], mybir.dt.float32, name=f"pagg{i}") for i in range(NPS)]

        NRING = 3
        gO = [sb.tile([128, G, D], mybir.dt.float32, name=f"gO{i}") for i in range(NRING)]
        gU = [sb.tile([128, G, D], mybir.dt.float32, name=f"gU{i}") for i in range(NRING)]
        ssr = [sb.tile([128, G], mybir.dt.float32, name=f"ssr{i}") for i in range(NRING)]
        sel = [sb.tile([128, msel, 128], mybir.dt.float32, name=f"sel{i}") for i in range(NRING)]

        NUB = 2
        ubr = [sb.tile([128, GB, D], mybir.dt.float32, name=f"ubr{i}") for i in range(NUB)]
        ubn = [sb.tile([128, GB, D], mybir.dt.float32, name=f"ubn{i}") for i in range(NUB)]
        rnb = [sb.tile([128, GB], mybir.dt.float32, name=f"rnb{i}") for i in range(NUB)]
        NPR = 2
        prod = [sb.tile([128, D], mybir.dt.float32, name=f"prod{i}") for i in range(NPR)]
        sums_sb = sb.tile([128, NB], mybir.dt.float32)
        nc.vector.memset(sums_sb[:], 0.0)

        n_ub_groups = (NB + GB - 1) // GB

        def load_ub_group(bg):
            b0 = bg * GB
            bn = min(GB, NB - b0)
            raw = ubr[bg % NUB]
            nrm = ubn[bg % NUB]
            rn = rnb[bg % NUB]
            for g in range(bn):
                r0 = (b0 + g) * 128
                nc.sync.dma_start(raw[:, g, :], feats[r0 : r0 + 128, :])
            nc.vector.tensor_tensor(
                out=nrm[:, :bn, :], in0=raw[:, :bn, :], in1=raw[:, :bn, :],
                op=mybir.AluOpType.mult,
            )
            nc.vector.tensor_reduce(
                out=rn[:, :bn], in_=nrm[:, :bn, :], op=mybir.AluOpType.add,
                axis=mybir.AxisListType.X,
            )
            nc.scalar.sqrt(out=rn[:, :bn], in_=rn[:, :bn])
            nc.vector.tensor_scalar_max(rn[:, :bn], rn[:, :bn], EPS)
            nc.vector.reciprocal(out=rn[:, :bn], in_=rn[:, :bn])
            nc.vector.tensor_tensor(
                out=nrm[:, :bn, :], in0=raw[:, :bn, :],
                in1=rn[:, :bn].unsqueeze(2).to_broadcast([128, bn, D]),
                op=mybir.AluOpType.mult,
            )

        load_ub_group(0)

        ngrp = (T + G - 1) // G
        for gi in range(ngrp):
            t0 = gi * G
            gn = min(G, T - t0)
            r = gi % NRING
            for g in range(gn):
                t = t0 + g
                nc.gpsimd.indirect_dma_start(
                    out=gO[r][:, g, :], out_offset=None, in_=feats[:],
                    in_offset=bass.IndirectOffsetOnAxis(ap=idx_oth_t[:, t : t + 1], axis=0),
                )
            # normalize gathered rows: gU = gO / max(||gO||_row, EPS)
            nc.vector.tensor_tensor(
                out=gU[r][:, :gn, :], in0=gO[r][:, :gn, :], in1=gO[r][:, :gn, :],
                op=mybir.AluOpType.mult,
            )
            nc.vector.tensor_reduce(
                out=ssr[r][:, :gn], in_=gU[r][:, :gn, :], op=mybir.AluOpType.add,
                axis=mybir.AxisListType.X,
            )
            nc.scalar.sqrt(out=ssr[r][:, :gn], in_=ssr[r][:, :gn])
            nc.vector.tensor_scalar_max(ssr[r][:, :gn], ssr[r][:, :gn], EPS)
            nc.vector.reciprocal(out=ssr[r][:, :gn], in_=ssr[r][:, :gn])
            nc.vector.tensor_tensor(
                out=gU[r][:, :gn, :], in0=gO[r][:, :gn, :],
                in1=ssr[r][:, :gn].unsqueeze(2).to_broadcast([128, gn, D]),
                op=mybir.AluOpType.mult,
            )
            s0, s1 = grp[gi]
            ns = s1 - s0
            nc.vector.tensor_tensor(
                out=sel[r][:, :ns, :],
                in0=locf_t[:, s0:s1].unsqueeze(2).to_broadcast([128, ns, 128]),
                in1=iota_t[:].to_broadcast([128, ns, 128]),
                op=mybir.AluOpType.is_equal,
            )
            for j in range(ns):
                t, b, fi, la = segs[s0 + j]
                nc.tensor.matmul(
                    out=pagg[b % NPS][:],
                    lhsT=sel[r][:, j, :],
                    rhs=gU[r][:, t - t0, :],
                    start=bool(fi),
                    stop=bool(la),
                )
                if la:
                    if b % GB == GB - 1 and (b + 1) // GB < n_ub_groups:
                        load_ub_group((b + 1) // GB)
                    pr = prod[b % NPR]
                    nc.vector.tensor_tensor(
                        out=pr[:], in0=pagg[b % NPS][:],
                        in1=ubn[(b // GB) % NUB][:, b % GB, :],
                        op=mybir.AluOpType.mult,
                    )
                    nc.vector.tensor_reduce(
                        out=sums_sb[:, b : b + 1], in_=pr[:].unsqueeze(1),
                        op=mybir.AluOpType.add, axis=mybir.AxisListType.X,
                    )

        nc.sync.dma_start(sums_out[:], sums_sb[:])

    nc.compile()
    return nc


def kernel(node_features: np.ndarray, edge_index: np.ndarray) -> np.ndarray:
    feats = np.zeros((NPAD, D), np.float32)
    feats[:N] = np.asarray(node_features, np.float32)

    meta, per_core = _build_metadata(np.asarray(edge_index))
    nc = _build_program(meta)
    global _LAST_NC
    _LAST_NC = nc

    iota = np.broadcast_to(np.arange(128, dtype=np.float32)[None, None, :], (128, 1, 128))
    iota = np.ascontiguousarray(iota)
    in_maps = [
        {
            "feats": feats,
            "idx_oth": pc["idx_oth"],
            "locf": pc["locf"],
            "iota": iota,
        }
        for pc in per_core
    ]
    res = run_bass_kernel_spmd(nc, in_maps, core_ids=list(range(NCORES)), trace=False)

    sums = np.zeros((128, NB), np.float64)
    for out in res.results:
        sums += out["sums_out"].astype(np.float64)
    sums_nodes = sums.T.ravel()[:N]

    tgt = np.concatenate([edge_index[0], edge_index[1]]).astype(np.int64)
    deg = np.bincount(tgt, minlength=N).astype(np.float64)
    out = np.where(deg > 0, sums_nodes / np.maximum(deg, 1.0), 1.0)
    return out.astype(np.float32)
